# revision 24
# baseline (speedup 1.0000x reference)
"""Trainium2 Bass kernel for nn_KernelMachine (random Fourier features).

out[n,m] = sum_f sqrt(2/F) * cos(x_n . a_f + b_f) * W[f*M+m]

Strategy (data-parallel over 8 NeuronCores, N sharded, a/b/W replicated):

Per core (N_loc=4096, D=16, F=4096, M=16), define t = (x.a + b + pi/2)/(2pi),
phi = sin(2*pi*t) = cos(x.a + b):

  1. m1 (PE, fp16, K=19): T = t + 1536.5 in PSUM fp32. All T land in
     [1024, 2048) => fixed exponent 2^10, frac(T) = mantissa bits 12..0.
  2. DVE ONE pass: y_bits = (T_bits & 0x1FFF) | 0x46000000, i.e.
     y = 8192 + 8*frac(T), fp32 SBUF (pure bit ops, exact).
  3. ACT Sin over 4-tile groups: phi = Sin((2pi/8)*y - 2pi*1024.5)
     = sin(2pi*frac(T) - pi) = sin(2pi*t) = cos(x.a+b).
     (The ACT affine is a true single-rounding FMA - verified.)
  4. m2 (PE, fp16, 2x column-tiled): out_ps[16 @ col-group h] +=
     wsc[:,c,:]^T @ phi[:, n-half h], accumulated over 32 f-chunks.
  5. epilogue: DVE 32x32 transpose + DMA out.

Load balance: the DVE bit pass (~1.19us/tile) is the bottleneck; ACT sin
at FD=4096 is ~0.9us/tile-equivalent. Every SHARE_PERIOD-th tile takes an
"ACT-share" path instead: ACT Identity(T + 2^23) = 2^23 + rint(T) (exact
magic round), PE corr matmul subtracts it in PSUM, ACT Sin reads PSUM
with scale=-2pi, bias=-2pi*2^23 (exact). This shifts ~11% of the
range-reduction work from DVE to ACT+PE, equalizing the drain engines.
"""

import math

import numpy as np

import concourse.bass as bass
import concourse.tile as tile
from concourse import bacc, mybir
from concourse.bass_utils import run_bass_kernel_spmd

F32 = mybir.dt.float32
U32 = mybir.dt.uint32
FP16 = mybir.dt.float16

N, D, F, M = 32768, 16, 4096, 16
NCORES = 8
NLOC = N // NCORES            # 4096 rows per core
FC = F // 128                 # 32 f-chunks of 128
NG = NLOC // 512              # 8 n-groups of 512

KROWS = D + 3                 # 16 a rows + [bh, bl, 1536] bias rows
K64 = 64                      # m1 contraction incl. cancelling noise pad
MW = 32                       # m2 stationary cols: 16 real W + 16 noise

TWO_PI_F32 = float(np.float32(2.0 * np.pi))
SIN_SCALE = float(np.float32(TWO_PI_F32 / 8.0))            # exact shift
SIN_BIAS = float(np.float32(-np.float64(TWO_PI_F32) * 1024.5))
MAGIC23 = float(2.0 ** 23)
SHARE_SCALE = float(np.float32(-TWO_PI_F32))
SHARE_BIAS = float(-np.float64(TWO_PI_F32) * (2.0 ** 23))  # exact fp32

W_PRESCALE = 1024.0
SHARE_PERIOD = 10 ** 6        # ACT-share path disabled (see note below)
RUN = 4                       # tiles per grouped Sin instruction
M2_LAG = 8                    # iterations between m1(i) and m2(i)
NDUMMY = 0                    # warm-keeper off: noise-padding suffices

_CACHE = {}


def build_nc():
    nc = bacc.Bacc(None, target_bir_lowering=False)

    x_in = nc.dram_tensor("x_in", [D, NLOC], FP16, kind="ExternalInput")
    apack0_in = nc.dram_tensor("apack0_in", [K64, 1024], FP16, kind="ExternalInput")
    apack1_in = nc.dram_tensor("apack1_in", [K64, F - 1024], FP16, kind="ExternalInput")
    wsc_in = nc.dram_tensor("wsc_in", [128, FC, MW], FP16, kind="ExternalInput")
    negi_in = nc.dram_tensor("negi_in", [128, 128], F32, kind="ExternalInput")
    ones_in = nc.dram_tensor("ones_in", [K64 - D, NLOC], FP16, kind="ExternalInput")
    out_t = nc.dram_tensor("out", [NLOC, M], F32, kind="ExternalOutput")

    NIT = (NG // 2) * FC      # 128 tiles: pair P=i//FC, f-chunk c=i%FC

    with tile.TileContext(nc) as tc:
        with (
            tc.tile_pool(name="const", bufs=1) as const,
            tc.tile_pool(name="ygp", bufs=5) as ygp,
            tc.tile_pool(name="ph4", bufs=6) as ph4,
            tc.tile_pool(name="ph1", bufs=3) as ph1,
            tc.tile_pool(name="kmp", bufs=2) as kmp,
            tc.tile_pool(name="osb", bufs=4) as osb,
            tc.tile_pool(name="pst", bufs=3, space="PSUM") as pst,
            tc.tile_pool(name="pso", bufs=1, space="PSUM") as pso,
        ):
            # ---------------- constants ----------------
            apack = const.tile([128, F], FP16, tag="apack")
            wsc = const.tile([128, FC, MW], FP16, tag="wsc")
            negi = const.tile([128, 128], F32, tag="negi")
            sin_bias = const.tile([128, 1], F32, tag="sinb")
            nc.gpsimd.memset(sin_bias, SIN_BIAS)
            magic_bias = const.tile([128, 1], F32, tag="magicb")
            nc.gpsimd.memset(magic_bias, MAGIC23)
            share_bias = const.tile([128, 1], F32, tag="shareb")
            nc.gpsimd.memset(share_bias, SHARE_BIAS)
            djw = const.tile([1, 16], FP16, tag="djw")
            nc.gpsimd.memset(djw, 0.0)
            djx = const.tile([1, 128], FP16, tag="djx")
            nc.gpsimd.memset(djx, 0.0)

            # ---------------- x load (host pre-transposed fp16) ----------
            xpack = const.tile([128, NLOC], FP16, tag="xpack")
            nc.sync.dma_start(out=xpack[0:D, 0:1024], in_=x_in[:, 0:1024])
            nc.sync.dma_start(out=xpack[64:64 + D, 0:1024], in_=x_in[:, 0:1024])
            nc.sync.dma_start(out=xpack[D:K64, 0:1024], in_=ones_in[:, 0:1024])
            nc.sync.dma_start(out=xpack[64 + D:128, 0:1024],
                              in_=ones_in[:, 0:1024])
            nc.sync.dma_start(out=apack[0:K64, 0:1024], in_=apack0_in[:])
            nc.sync.dma_start(out=apack[64:128, 0:1024], in_=apack0_in[:])
            nc.sync.dma_start(out=xpack[0:D, 1024:], in_=x_in[:, 1024:])
            nc.sync.dma_start(out=xpack[64:64 + D, 1024:], in_=x_in[:, 1024:])
            nc.sync.dma_start(out=apack[0:K64, 1024:], in_=apack1_in[:])
            nc.sync.dma_start(out=apack[64:128, 1024:], in_=apack1_in[:])
            nc.sync.dma_start(out=xpack[D:K64, 1024:], in_=ones_in[:, 1024:])
            nc.sync.dma_start(out=xpack[64 + D:128, 1024:],
                              in_=ones_in[:, 1024:])
            nc.sync.dma_start(out=wsc, in_=wsc_in[:])
            nc.sync.dma_start(out=negi, in_=negi_in[:])

            # ---------------- main loop state ----------------
            is_share = [(i % SHARE_PERIOD == SHARE_PERIOD - 1)
                        for i in range(NIT)]

            t_tiles = {}
            km_tiles = {}
            out_ps_by_p = {}
            phi_of = {}           # tile idx -> (phi_tile, off | None)
            ygrp = [None, 0, []]  # current [tile, fill_count, member_ids]

            def emit_m1(i):
                P, c = divmod(i, FC)
                tp = pst.tile([128, 1024], F32, tag="t")
                for h in range(2):
                    g = 2 * P + h
                    rq = 64 * h
                    nc.tensor.matmul(
                        tp[:, 512 * h:512 * (h + 1)],
                        apack[rq:rq + K64, 128 * c:128 * (c + 1)],
                        xpack[rq:rq + K64, 512 * g:512 * (g + 1)],
                        start=True, stop=(not is_share[i]),
                        tile_position=(rq, 0),
                    )
                t_tiles[i] = tp

            def flush_ring_sin():
                yt, cnt, members = ygrp
                if cnt == 0:
                    return
                phi = ph4.tile([128, RUN, 1024], FP16, tag="phi4")
                nc.scalar.activation(
                    out=phi[:, 0:cnt, :], in_=yt[:, 0:cnt, :],
                    func=mybir.ActivationFunctionType.Sin,
                    bias=sin_bias[:, 0:1], scale=SIN_SCALE)
                for j, idx in enumerate(members):
                    phi_of[idx] = (phi, j)
                ygrp[0], ygrp[1] = None, 0
                ygrp[2] = []

            def emit_reduce(i):
                tp = t_tiles[i]
                if not is_share[i]:
                    if ygrp[0] is None:
                        ygrp[0] = ygp.tile([128, RUN, 1024], F32,
                                           name="ygt", tag="ygrp")
                    j = ygrp[1]
                    nc.vector.tensor_scalar(
                        out=ygrp[0][:, j, :].bitcast(U32),
                        in0=tp[:].bitcast(U32),
                        scalar1=0x00001FFF, scalar2=0x46000000,
                        op0=mybir.AluOpType.bitwise_and,
                        op1=mybir.AluOpType.bitwise_or)
                    t_tiles.pop(i)
                    ygrp[1] += 1
                    ygrp[2].append(i)
                    if ygrp[1] == RUN:
                        flush_ring_sin()
                else:
                    km = kmp.tile([128, 1024], F32, tag="km")
                    nc.scalar.activation(
                        out=km, in_=tp[:],
                        func=mybir.ActivationFunctionType.Identity,
                        bias=magic_bias[:, 0:1], scale=1.0)
                    km_tiles[i] = km

            def emit_corr_sin_share(i):
                tp = t_tiles.pop(i)
                km = km_tiles.pop(i)
                for h in range(2):
                    nc.tensor.matmul(
                        tp[:, 512 * h:512 * (h + 1)],
                        negi,
                        km[:, 512 * h:512 * (h + 1)],
                        start=False, stop=True,
                    )
                phi = ph1.tile([128, 1024], FP16, tag="phi1")
                nc.scalar.activation(
                    out=phi, in_=tp[:],
                    func=mybir.ActivationFunctionType.Sin,
                    bias=share_bias[:, 0:1], scale=SHARE_SCALE)
                phi_of[i] = (phi, None)

            def emit_epilogue(P):
                out_ps = out_ps_by_p.pop(P)
                for h in range(2):
                    g = 2 * P + h
                    outT = osb.tile([16, 512], F32, tag="outT16")
                    nc.scalar.mul(outT,
                                  out_ps[32 * h:32 * h + 16,
                                         512 * h:512 * (h + 1)],
                                  1.0 / W_PRESCALE)
                    # transpose on the DRAM side of the DMA (strided
                    # writes) - keeps the DVE out of the epilogue
                    nc.sync.dma_start(
                        out=out_t[512 * g:512 * (g + 1), :].rearrange(
                            "n m -> m n"),
                        in_=outT,
                    )

            def emit_m2(i):
                P, c = divmod(i, FC)
                if c == 0:
                    out_ps = pso.tile([96, 1024], F32, tag="o")
                    out_ps_by_p[P] = out_ps
                out_ps = out_ps_by_p[P]
                phi, off = phi_of.pop(i)
                for h in range(2):
                    if off is None:
                        rhs = phi[:, 512 * h:512 * (h + 1)]
                    else:
                        rhs = phi[:, off, 512 * h:512 * (h + 1)]
                    # col-group h accumulates in its OWN psum bank (column
                    # offset 512*h): sharing one bank across col-groups
                    # corrupts the accumulation (measured). M=32 = 16 real
                    # W cols + 16 noise cols for PE array activity (HAM).
                    nc.tensor.matmul(
                        out_ps[32 * h:32 * h + 32, 512 * h:512 * (h + 1)],
                        wsc[:, c, :],
                        rhs,
                        start=(c == 0), stop=(c == FC - 1),
                        tile_position=(0, 32 * h),
                    )
                if c == FC - 1:
                    emit_epilogue(P)

            # ---------------- software pipeline ----------------
            # Warm-keeper: tiny dependency-free matmuls into the scratch
            # partitions (64:80) of the out psum banks. They keep the PE
            # HAM activity monitor busy so the clock gate stays at 8/8
            # (2.4 GHz); without them the PE micro-idles, gets throttled
            # to 1.2 GHz, and becomes the critical path. start=False so
            # the real accumulation bits of the bank are never cleared.
            def emit_dummy(n=1):
                out_ps = out_ps_by_p.get(max(out_ps_by_p)) if out_ps_by_p \
                    else None
                if out_ps is None:
                    return
                for _ in range(n):
                    nc.tensor.matmul(
                        out_ps[64:96, 0:512], apack[0:64, 0:32],
                        xpack[0:64, 0:512],
                        start=False, stop=True,
                        tile_position=(0, 64),
                        skip_group_check=True,
                    )

            for it in range(NIT + M2_LAG):
                if it < NIT:
                    emit_m1(it)
                if NDUMMY:
                    emit_dummy(NDUMMY)
                if 0 <= it - 1 < NIT:
                    emit_reduce(it - 1)
                    if it - 1 == NIT - 1 and ygrp[1] > 0:
                        flush_ring_sin()
                if 0 <= it - 2 < NIT and is_share[it - 2]:
                    emit_corr_sin_share(it - 2)
                if 0 <= it - M2_LAG < NIT:
                    emit_m2(it - M2_LAG)

    nc.finalize()
    return nc


def _host_prep(a, b, W):
    """Precompute replicated operand packs."""
    inv2pi = 1.0 / (2.0 * np.pi)
    a64 = np.asarray(a, dtype=np.float64).T * inv2pi          # [16, F]
    b64 = (np.asarray(b, dtype=np.float64) + np.pi / 2.0) * inv2pi + 0.5
    bh = b64.astype(np.float16)
    bl = (b64 - bh.astype(np.float64)).astype(np.float16)

    rng = np.random.default_rng(12345)
    NPAD = K64 - D - 4            # 44 cancelling noise rows (22 pairs)
    # apack rows: [a' (0:16), pad v-pairs (16:60), bh, bl, 1536, 0]
    apack = np.zeros((K64, F), dtype=np.float16)
    apack[0:D] = a64.astype(np.float16)
    vp = rng.uniform(0.05, 0.2, size=(NPAD // 2, F)).astype(np.float16)
    vs = np.sign(rng.normal(size=(NPAD // 2, F))).astype(np.float16)
    vpair = (vp * vs)
    apack[D:D + NPAD:2] = vpair
    apack[D + 1:D + NPAD + 1:2] = vpair      # identical v in each pair
    apack[D + NPAD] = bh
    apack[D + NPAD + 1] = bl
    apack[D + NPAD + 2] = 1536.0
    # xpack tail rows D:K64 = [r, -r pairs (44), 1, 1, 1, 0]
    ones = np.zeros((K64 - D, NLOC), dtype=np.float16)
    rp = (rng.uniform(0.05, 0.2, size=(NPAD // 2, NLOC)) *
          np.sign(rng.normal(size=(NPAD // 2, NLOC)))).astype(np.float16)
    ones[0:NPAD:2] = rp
    ones[1:NPAD + 1:2] = -rp                 # exact cancellation per pair
    ones[NPAD:NPAD + 3] = 1.0

    scale = math.sqrt(2.0 / F)
    W2 = (np.asarray(W, dtype=np.float64).reshape(F, M) * scale * W_PRESCALE
          ).astype(np.float16)
    W2 = np.concatenate(
        [W2, (rng.uniform(0.02, 0.1, size=(F, MW - M)) *
              np.sign(rng.normal(size=(F, MW - M)))).astype(np.float16)],
        axis=1)                               # noise W cols for activity
    wsc = np.ascontiguousarray(
        W2.reshape(FC, 128, MW).transpose(1, 0, 2)
    )                                                          # [128, FC, MW]

    negi = (-np.eye(128)).astype(np.float32)
    return apack, wsc, negi, ones


def kernel(x, a, b, W):
    xT = np.ascontiguousarray(
        np.asarray(x, dtype=np.float32).T.astype(np.float16))  # [16, N]
    apack, wsc, negi, ones = _host_prep(a, b, W)

    if "nc" not in _CACHE:
        _CACHE["nc"] = build_nc()
    nc = _CACHE["nc"]

    in_maps = []
    for i in range(NCORES):
        in_maps.append({
            "x_in": np.ascontiguousarray(xT[:, i * NLOC:(i + 1) * NLOC]),
            "apack0_in": np.ascontiguousarray(apack[:, 0:1024]),
            "apack1_in": np.ascontiguousarray(apack[:, 1024:]),
            "wsc_in": wsc,
            "negi_in": negi,
            "ones_in": ones,
        })

    res = run_bass_kernel_spmd(nc, in_maps, core_ids=list(range(NCORES)))
    return np.concatenate([r["out"] for r in res.results], axis=0)


# revision 25
# speedup vs baseline: 1.4735x; 1.4735x over previous
"""Trainium2 Bass kernel for nn_KernelMachine (random Fourier features).

out[n,m] = sum_f sqrt(2/F) * cos(x_n . a_f + b_f) * W[f*M+m]

Strategy (data-parallel over 8 NeuronCores, N sharded, a/b/W replicated):

Per core (N_loc=4096, D=16, F=4096, M=16), define t = (x.a + b + pi/2)/(2pi),
phi = sin(2*pi*t) = cos(x.a + b):

  1. m1 (PE, fp16, K=19): T = t + 1536.5 in PSUM fp32. All T land in
     [1024, 2048) => fixed exponent 2^10, frac(T) = mantissa bits 12..0.
  2. DVE ONE pass: y_bits = (T_bits & 0x1FFF) | 0x46000000, i.e.
     y = 8192 + 8*frac(T), fp32 SBUF (pure bit ops, exact).
  3. ACT Sin over 4-tile groups: phi = Sin((2pi/8)*y - 2pi*1024.5)
     = sin(2pi*frac(T) - pi) = sin(2pi*t) = cos(x.a+b).
     (The ACT affine is a true single-rounding FMA - verified.)
  4. m2 (PE, fp16, 2x column-tiled): out_ps[16 @ col-group h] +=
     wsc[:,c,:]^T @ phi[:, n-half h], accumulated over 32 f-chunks.
  5. epilogue: DVE 32x32 transpose + DMA out.

Load balance: the DVE bit pass (~1.19us/tile) is the bottleneck; ACT sin
at FD=4096 is ~0.9us/tile-equivalent. Every SHARE_PERIOD-th tile takes an
"ACT-share" path instead: ACT Identity(T + 2^23) = 2^23 + rint(T) (exact
magic round), PE corr matmul subtracts it in PSUM, ACT Sin reads PSUM
with scale=-2pi, bias=-2pi*2^23 (exact). This shifts ~11% of the
range-reduction work from DVE to ACT+PE, equalizing the drain engines.
"""

import math

import numpy as np

import concourse.bass as bass
import concourse.tile as tile
from concourse import bacc, mybir
from concourse.bass_utils import run_bass_kernel_spmd

F32 = mybir.dt.float32
U32 = mybir.dt.uint32
FP16 = mybir.dt.float16

N, D, F, M = 32768, 16, 4096, 16
NCORES = 8
NLOC = N // NCORES            # 4096 rows per core
FC = F // 128                 # 32 f-chunks of 128
NG = NLOC // 512              # 8 n-groups of 512

KROWS = D + 3                 # 16 a rows + [bh, bl, 1536] bias rows
K64 = 64                      # m1 contraction incl. cancelling noise pad
MW = 32                       # m2 stationary cols: 16 real W + 16 noise

TWO_PI_F32 = float(np.float32(2.0 * np.pi))
SIN_SCALE = float(np.float32(TWO_PI_F32 / 8.0))            # exact shift
SIN_BIAS = float(np.float32(-np.float64(TWO_PI_F32) * 1024.5))
MAGIC23 = float(2.0 ** 23)
SHARE_SCALE = float(np.float32(-TWO_PI_F32))
SHARE_BIAS = float(-np.float64(TWO_PI_F32) * (2.0 ** 23))  # exact fp32

W_PRESCALE = 1024.0
SHARE_PERIOD = 10 ** 6        # ACT-share path disabled (see note below)
RUN = 4                       # tiles per grouped Sin instruction
M2_LAG = 8                    # iterations between m1(i) and m2(i)
NDUMMY = 0                    # warm-keeper off: noise-padding suffices

_CACHE = {}


def build_nc():
    nc = bacc.Bacc(None, target_bir_lowering=False)

    x_in = nc.dram_tensor("x_in", [D, NLOC], FP16, kind="ExternalInput")
    apack0_in = nc.dram_tensor("apack0_in", [K64, 1024], FP16, kind="ExternalInput")
    apack1_in = nc.dram_tensor("apack1_in", [K64, F - 1024], FP16, kind="ExternalInput")
    wsc_in = nc.dram_tensor("wsc_in", [128, FC, MW], FP16, kind="ExternalInput")
    negi_in = nc.dram_tensor("negi_in", [128, 128], F32, kind="ExternalInput")
    ones_in = nc.dram_tensor("ones_in", [K64 - D, NLOC], FP16, kind="ExternalInput")
    out_t = nc.dram_tensor("out", [NLOC, M], F32, kind="ExternalOutput")

    NIT = (NG // 2) * FC      # 128 tiles: pair P=i//FC, f-chunk c=i%FC

    with tile.TileContext(nc) as tc:
        with (
            tc.tile_pool(name="const", bufs=1) as const,
            tc.tile_pool(name="ygp", bufs=5) as ygp,
            tc.tile_pool(name="ph4", bufs=6) as ph4,
            tc.tile_pool(name="ph1", bufs=3) as ph1,
            tc.tile_pool(name="kmp", bufs=2) as kmp,
            tc.tile_pool(name="osb", bufs=4) as osb,
            tc.tile_pool(name="pst", bufs=3, space="PSUM") as pst,
            tc.tile_pool(name="pso", bufs=1, space="PSUM") as pso,
        ):
            # ---------------- constants ----------------
            apack = const.tile([128, F], FP16, tag="apack")
            wsc = const.tile([128, FC, MW], FP16, tag="wsc")
            negi = const.tile([128, 128], F32, tag="negi")
            sin_bias = const.tile([128, 1], F32, tag="sinb")
            nc.gpsimd.memset(sin_bias, SIN_BIAS)
            magic_bias = const.tile([128, 1], F32, tag="magicb")
            nc.gpsimd.memset(magic_bias, MAGIC23)
            share_bias = const.tile([128, 1], F32, tag="shareb")
            nc.gpsimd.memset(share_bias, SHARE_BIAS)
            djw = const.tile([1, 16], FP16, tag="djw")
            nc.gpsimd.memset(djw, 0.0)
            djx = const.tile([1, 128], FP16, tag="djx")
            nc.gpsimd.memset(djx, 0.0)

            # ---------------- x load (host pre-transposed fp16) ----------
            xpack = const.tile([128, NLOC], FP16, tag="xpack")
            nc.sync.dma_start(out=xpack[0:D, 0:1024], in_=x_in[:, 0:1024])
            nc.sync.dma_start(out=xpack[64:64 + D, 0:1024], in_=x_in[:, 0:1024])
            nc.sync.dma_start(out=xpack[D:K64, 0:1024], in_=ones_in[:, 0:1024])
            nc.sync.dma_start(out=xpack[64 + D:128, 0:1024],
                              in_=ones_in[:, 0:1024])
            nc.sync.dma_start(out=apack[0:K64, 0:1024], in_=apack0_in[:])
            nc.sync.dma_start(out=apack[64:128, 0:1024], in_=apack0_in[:])
            nc.sync.dma_start(out=xpack[0:D, 1024:], in_=x_in[:, 1024:])
            nc.sync.dma_start(out=xpack[64:64 + D, 1024:], in_=x_in[:, 1024:])
            nc.sync.dma_start(out=apack[0:K64, 1024:], in_=apack1_in[:])
            nc.sync.dma_start(out=apack[64:128, 1024:], in_=apack1_in[:])
            nc.sync.dma_start(out=xpack[D:K64, 1024:], in_=ones_in[:, 1024:])
            nc.sync.dma_start(out=xpack[64 + D:128, 1024:],
                              in_=ones_in[:, 1024:])
            nc.sync.dma_start(out=wsc, in_=wsc_in[:])
            nc.sync.dma_start(out=negi, in_=negi_in[:])

            # ---------------- main loop state ----------------
            is_share = [(i % SHARE_PERIOD == SHARE_PERIOD - 1)
                        for i in range(NIT)]

            t_tiles = {}
            km_tiles = {}
            out_ps_by_p = {}
            phi_of = {}           # tile idx -> (phi_tile, off | None)
            ygrp = [None, 0, []]  # current [tile, fill_count, member_ids]

            def emit_m1(i):
                P, c = divmod(i, FC)
                tp = pst.tile([128, 1024], F32, tag="t")
                for h in range(2):
                    g = 2 * P + h
                    rq = 64 * h
                    nc.tensor.matmul(
                        tp[:, 512 * h:512 * (h + 1)],
                        apack[rq:rq + K64, 128 * c:128 * (c + 1)],
                        xpack[rq:rq + K64, 512 * g:512 * (g + 1)],
                        start=True, stop=(not is_share[i]),
                        tile_position=(rq, 0),
                    )
                t_tiles[i] = tp

            def flush_ring_sin():
                yt, cnt, members = ygrp
                if cnt == 0:
                    return
                phi = ph4.tile([128, RUN, 1024], FP16, tag="phi4")
                nc.scalar.activation(
                    out=phi[:, 0:cnt, :], in_=yt[:, 0:cnt, :],
                    func=mybir.ActivationFunctionType.Sin,
                    bias=sin_bias[:, 0:1], scale=SIN_SCALE)
                for j, idx in enumerate(members):
                    phi_of[idx] = (phi, j)
                ygrp[0], ygrp[1] = None, 0
                ygrp[2] = []

            def emit_reduce(i):
                tp = t_tiles[i]
                if not is_share[i]:
                    if ygrp[0] is None:
                        ygrp[0] = ygp.tile([128, RUN, 1024], F32,
                                           name="ygt", tag="ygrp")
                    j = ygrp[1]
                    nc.vector.tensor_scalar(
                        out=ygrp[0][:, j, :].bitcast(U32),
                        in0=tp[:].bitcast(U32),
                        scalar1=0x00001FFF, scalar2=0x46000000,
                        op0=mybir.AluOpType.bitwise_and,
                        op1=mybir.AluOpType.bitwise_or)
                    t_tiles.pop(i)
                    ygrp[1] += 1
                    ygrp[2].append(i)
                    if ygrp[1] == RUN:
                        flush_ring_sin()
                else:
                    km = kmp.tile([128, 1024], F32, tag="km")
                    nc.scalar.activation(
                        out=km, in_=tp[:],
                        func=mybir.ActivationFunctionType.Identity,
                        bias=magic_bias[:, 0:1], scale=1.0)
                    km_tiles[i] = km

            def emit_corr_sin_share(i):
                tp = t_tiles.pop(i)
                km = km_tiles.pop(i)
                for h in range(2):
                    nc.tensor.matmul(
                        tp[:, 512 * h:512 * (h + 1)],
                        negi,
                        km[:, 512 * h:512 * (h + 1)],
                        start=False, stop=True,
                    )
                phi = ph1.tile([128, 1024], FP16, tag="phi1")
                nc.scalar.activation(
                    out=phi, in_=tp[:],
                    func=mybir.ActivationFunctionType.Sin,
                    bias=share_bias[:, 0:1], scale=SHARE_SCALE)
                phi_of[i] = (phi, None)

            def emit_epilogue(P):
                out_ps = out_ps_by_p.pop(P)
                for h in range(2):
                    g = 2 * P + h
                    outT = osb.tile([32, 512], F32, tag="outT")
                    nc.gpsimd.memset(outT, 0.0)
                    nc.scalar.mul(outT[0:16, :],
                                  out_ps[32 * h:32 * h + 16,
                                         512 * h:512 * (h + 1)],
                                  1.0 / W_PRESCALE)
                    blockT = osb.tile([32, 512], F32, tag="blockT")
                    nc.vector.transpose(out=blockT, in_=outT)
                    nc.sync.dma_start(
                        out=out_t[512 * g:512 * (g + 1), :].rearrange(
                            "(cb i) m -> i cb m", i=32
                        ),
                        in_=blockT.rearrange(
                            "p (cb jj) -> p cb jj", jj=32)[:, :, 0:M],
                    )

            def emit_m2(i):
                P, c = divmod(i, FC)
                if c == 0:
                    out_ps = pso.tile([96, 1024], F32, tag="o")
                    out_ps_by_p[P] = out_ps
                out_ps = out_ps_by_p[P]
                phi, off = phi_of.pop(i)
                for h in range(2):
                    if off is None:
                        rhs = phi[:, 512 * h:512 * (h + 1)]
                    else:
                        rhs = phi[:, off, 512 * h:512 * (h + 1)]
                    # col-group h accumulates in its OWN psum bank (column
                    # offset 512*h): sharing one bank across col-groups
                    # corrupts the accumulation (measured). M=32 = 16 real
                    # W cols + 16 noise cols for PE array activity (HAM).
                    nc.tensor.matmul(
                        out_ps[32 * h:32 * h + 32, 512 * h:512 * (h + 1)],
                        wsc[:, c, :],
                        rhs,
                        start=(c == 0), stop=(c == FC - 1),
                        tile_position=(0, 32 * h),
                    )
                if c == FC - 1:
                    emit_epilogue(P)

            # ---------------- software pipeline ----------------
            # Warm-keeper: tiny dependency-free matmuls into the scratch
            # partitions (64:80) of the out psum banks. They keep the PE
            # HAM activity monitor busy so the clock gate stays at 8/8
            # (2.4 GHz); without them the PE micro-idles, gets throttled
            # to 1.2 GHz, and becomes the critical path. start=False so
            # the real accumulation bits of the bank are never cleared.
            def emit_dummy(n=1):
                out_ps = out_ps_by_p.get(max(out_ps_by_p)) if out_ps_by_p \
                    else None
                if out_ps is None:
                    return
                for _ in range(n):
                    nc.tensor.matmul(
                        out_ps[64:96, 0:512], apack[0:64, 0:32],
                        xpack[0:64, 0:512],
                        start=False, stop=True,
                        tile_position=(0, 64),
                        skip_group_check=True,
                    )

            for it in range(NIT + M2_LAG):
                if it < NIT:
                    emit_m1(it)
                if NDUMMY:
                    emit_dummy(NDUMMY)
                if 0 <= it - 1 < NIT:
                    emit_reduce(it - 1)
                    if it - 1 == NIT - 1 and ygrp[1] > 0:
                        flush_ring_sin()
                if 0 <= it - 2 < NIT and is_share[it - 2]:
                    emit_corr_sin_share(it - 2)
                if 0 <= it - M2_LAG < NIT:
                    emit_m2(it - M2_LAG)

    nc.finalize()
    return nc


def _host_prep(a, b, W):
    """Precompute replicated operand packs."""
    inv2pi = 1.0 / (2.0 * np.pi)
    a64 = np.asarray(a, dtype=np.float64).T * inv2pi          # [16, F]
    b64 = (np.asarray(b, dtype=np.float64) + np.pi / 2.0) * inv2pi + 0.5
    bh = b64.astype(np.float16)
    bl = (b64 - bh.astype(np.float64)).astype(np.float16)

    rng = np.random.default_rng(12345)
    NPAD = K64 - D - 4            # 44 cancelling noise rows (22 pairs)
    # apack rows: [a' (0:16), pad v-pairs (16:60), bh, bl, 1536, 0]
    apack = np.zeros((K64, F), dtype=np.float16)
    apack[0:D] = a64.astype(np.float16)
    vp = rng.uniform(0.05, 0.2, size=(NPAD // 2, F)).astype(np.float16)
    vs = np.sign(rng.normal(size=(NPAD // 2, F))).astype(np.float16)
    vpair = (vp * vs)
    apack[D:D + NPAD:2] = vpair
    apack[D + 1:D + NPAD + 1:2] = vpair      # identical v in each pair
    apack[D + NPAD] = bh
    apack[D + NPAD + 1] = bl
    apack[D + NPAD + 2] = 1536.0
    # xpack tail rows D:K64 = [r, -r pairs (44), 1, 1, 1, 0]
    ones = np.zeros((K64 - D, NLOC), dtype=np.float16)
    rp = (rng.uniform(0.05, 0.2, size=(NPAD // 2, NLOC)) *
          np.sign(rng.normal(size=(NPAD // 2, NLOC)))).astype(np.float16)
    ones[0:NPAD:2] = rp
    ones[1:NPAD + 1:2] = -rp                 # exact cancellation per pair
    ones[NPAD:NPAD + 3] = 1.0

    scale = math.sqrt(2.0 / F)
    W2 = (np.asarray(W, dtype=np.float64).reshape(F, M) * scale * W_PRESCALE
          ).astype(np.float16)
    W2 = np.concatenate(
        [W2, (rng.uniform(0.02, 0.1, size=(F, MW - M)) *
              np.sign(rng.normal(size=(F, MW - M)))).astype(np.float16)],
        axis=1)                               # noise W cols for activity
    wsc = np.ascontiguousarray(
        W2.reshape(FC, 128, MW).transpose(1, 0, 2)
    )                                                          # [128, FC, MW]

    negi = (-np.eye(128)).astype(np.float32)
    return apack, wsc, negi, ones


def kernel(x, a, b, W):
    xT = np.ascontiguousarray(
        np.asarray(x, dtype=np.float32).T.astype(np.float16))  # [16, N]
    apack, wsc, negi, ones = _host_prep(a, b, W)

    if "nc" not in _CACHE:
        _CACHE["nc"] = build_nc()
    nc = _CACHE["nc"]

    in_maps = []
    for i in range(NCORES):
        in_maps.append({
            "x_in": np.ascontiguousarray(xT[:, i * NLOC:(i + 1) * NLOC]),
            "apack0_in": np.ascontiguousarray(apack[:, 0:1024]),
            "apack1_in": np.ascontiguousarray(apack[:, 1024:]),
            "wsc_in": wsc,
            "negi_in": negi,
            "ones_in": ones,
        })

    res = run_bass_kernel_spmd(nc, in_maps, core_ids=list(range(NCORES)))
    return np.concatenate([r["out"] for r in res.results], axis=0)
